# revision 41
# baseline (speedup 1.0000x reference)
"""CodeGen attention block (B=1, S=2048, E=2048, H=16, D=128, rot=64) on 8 TRN2
NeuronCores.

Sharding: tensor-parallel over heads (2 heads/core). Each core computes its
heads' q/k in transposed [d, s] layout (partial rotary applied via a host-side
even/odd channel permutation folded into the qkv weights), v in [s, d] layout,
causal softmax attention entirely on-chip (scores transposed [k, q]; softmax
denominators accumulated on the vector engine + one ones-matmul partition
reduce), then computes a PARTIAL output projection over its 256 local features
for ALL 2048 output channels, and ReduceScatters the partials so each core
lands its own 256-row slice of out^T. No AllGather and no PE dependency on any
collective: the RS chain trails the compute.

All PE-facing tensors are bf16; accumulation is fp32 in PSUM. Attention /
RS sub-chunks: (512, 512, 512, 384, 128) wide in q; the narrow final chunks
shrink the end-of-kernel collective tail. Score matmuls are emitted with a
lookahead-2 software pipeline so PV matmuls don't stall on exp+mask; the
causal mask multiply only touches the 128-col triangular band of diagonal
tiles. Initial hidden/weight loads are spread over 4 engine DMA queues,
ordered so chunk j=0's operands (+qkv weights) land first.
"""

import numpy as np

H, D, ROT, MP = 16, 128, 64, 4
S, E = 2048, 2048
NCORES = 8
P = 128
NQ = 4            # 512-wide q chunks for the qkv projection
NEC = E // P      # 16 contraction chunks
SCALE = float(1.0 / np.sqrt(np.float64(D)))

# attention / AllGather sub-chunks: (q_start, width)
SUBS = [(0, 512), (512, 512), (1024, 512), (1536, 384), (1920, 128)]
# which sub-chunks run after qkv chunk j
SUBS_OF_J = {0: [0], 1: [1], 2: [2], 3: [3, 4]}

_CACHE = {}


# ----------------------------------------------------------------------------
# host-side input prep
# ----------------------------------------------------------------------------

def _head_rows(h):
    g, j = h // 4, h % 4
    base = g * (3 * 512)
    q = np.arange(base + j * 128, base + (j + 1) * 128)
    v = np.arange(base + 512 + j * 128, base + 512 + (j + 1) * 128)
    k = np.arange(base + 1024 + j * 128, base + 1024 + (j + 1) * 128)
    return q, v, k


def _prep_core_weights(c, w_qkv, w_out):
    h0, h1 = 2 * c, 2 * c + 1
    top = np.arange(0, ROT, 2)
    bot = np.arange(1, ROT, 2)
    wq, wk, wv = {}, {}, {}
    for h in (h0, h1):
        qr, vr, kr = _head_rows(h)
        wq[h], wv[h], wk[h] = w_qkv[qr], w_qkv[vr], w_qkv[kr]
    G0 = np.concatenate([wq[h0][top], wq[h1][top], wk[h0][top], wk[h1][top]], 0)
    G1 = np.concatenate([wq[h0][bot], wq[h1][bot], wk[h0][bot], wk[h1][bot]], 0)
    G2 = np.concatenate([wq[h0][ROT:], wq[h1][ROT:]], 0)
    G3 = np.concatenate([wk[h0][ROT:], wk[h1][ROT:]], 0)
    wqkT = np.concatenate([G0, G1, G2, G3], 0).T                       # [E, 512]
    wvT = np.concatenate([wv[h0], wv[h1]], 0).T                        # [E, 256]
    # single [E, 768] tensor: qk block then v block -> one DMA per row-chunk
    wqkvT = np.ascontiguousarray(
        np.concatenate([wqkT, wvT], 1), dtype=np.float32)              # [E, 768]
    woutT = np.ascontiguousarray(
        w_out[256 * c:256 * (c + 1), :].T, dtype=np.float32)           # [E, 256]
    # pre-pack for a fully contiguous [128, 16, 256] SBUF load
    woutT = np.ascontiguousarray(
        woutT.reshape(16, 128, 256).transpose(1, 0, 2)).reshape(128, 16 * 256)
    return wqkvT, woutT


def _cos_sin():
    inv_freq = 1.0 / (10000.0 ** (np.arange(0, ROT, 2, dtype=np.float32) / ROT))
    ang = np.arange(S, dtype=np.float32)[:, None] * inv_freq[None, :]
    cosb = np.cos(ang).T.astype(np.float32)        # [32, S]
    sinb = np.sin(ang).T.astype(np.float32)
    return (np.ascontiguousarray(np.tile(cosb, (4, 1))),
            np.ascontiguousarray(np.tile(sinb, (4, 1))))               # [128, S]


def _mask_tiles():
    m = np.zeros((128, 4 * 512), dtype=np.float32)
    pp = np.arange(128)[:, None]
    cc = np.arange(512)[None, :]
    for mm in range(4):
        m[:, mm * 512:(mm + 1) * 512] = ((pp + 128 * mm) <= cc).astype(np.float32)
    return m


def _build_in_maps(hidden_states, w_qkv, w_out):
    import ml_dtypes
    bf16 = ml_dtypes.bfloat16
    hiddenT = np.ascontiguousarray(
        np.asarray(hidden_states, np.float32).reshape(S, E).T).astype(bf16)
    COS, SIN = _cos_sin()
    COS, SIN = COS.astype(bf16), SIN.astype(bf16)
    masks = _mask_tiles().astype(bf16)
    in_maps = []
    for c in range(NCORES):
        wqkvT, wo = _prep_core_weights(
            c, np.asarray(w_qkv, np.float32), np.asarray(w_out, np.float32))
        in_maps.append({
            "hiddenT": hiddenT,
            "wqkvT": wqkvT.astype(bf16),
            "woutT": wo.astype(bf16),
            "cosT": COS,
            "sinT": SIN,
            "masks": masks,
        })
    return in_maps


# ----------------------------------------------------------------------------
# device program
# ----------------------------------------------------------------------------

def _kernel_body(tc, outT, hiddenT, wqkvT, woutT, cosT, sinT, masksT):
    import concourse.mybir as mybir
    from contextlib import ExitStack

    nc = tc.nc
    f32 = mybir.dt.float32
    bt = mybir.dt.bfloat16

    with ExitStack() as ctx:
        const = ctx.enter_context(tc.tile_pool(name="const", bufs=1))
        mask_sb = const.tile([P, 4 * 512], bt, name="mask_sb")
        ones_sb = const.tile([P, P], bt, name="ones_sb")
        wo_sb = const.tile([P, NEC, 256], bt, name="wo_sb")
        nc.vector.memset(ones_sb[:], 1.0)

        # one DRAM pool PER tensor: tiles sharing a pool share one memory
        # location set, which would add false dependencies serializing
        # the collective chain.
        ag_in, ag_out = [], []
        for s, (_, w) in enumerate(SUBS):
            pi = ctx.enter_context(
                tc.tile_pool(name=f"dri{s}", bufs=1, space="DRAM"))
            po = ctx.enter_context(
                tc.tile_pool(name=f"dro{s}", bufs=1, space="DRAM"))
            ag_in.append(pi.tile([2 * P, w], bt, name=f"ag_in{s}",
                                 tag=f"ag_in{s}"))
            ag_out.append(po.tile([E, w], bt, name=f"ag_out{s}",
                                  tag=f"ag_out{s}", addr_space="Shared"))

        with tc.tile_pool(name="ph1c", bufs=1) as ph1c_pool, \
             tc.tile_pool(name="chunk", bufs=1) as ck_pool, \
             tc.tile_pool(name="hid", bufs=2) as hid_pool, \
             tc.tile_pool(name="oti", bufs=1) as oti_pool, \
             tc.tile_pool(name="wts", bufs=1) as wts_pool, \
             tc.tile_pool(name="g01c", bufs=2) as g01_pool, \
             tc.tile_pool(name="rtmp", bufs=1) as rtmp_pool, \
             tc.tile_pool(name="pt", bufs=5) as pt_pool, \
             tc.tile_pool(name="dac", bufs=2) as dac_pool, \
             tc.tile_pool(name="den", bufs=2) as den_pool, \
             tc.tile_pool(name="otn", bufs=4) as otn_pool, \
             tc.tile_pool(name="psb", bufs=3) as psb_pool, \
             tc.tile_pool(name="gps", bufs=2, space="PSUM") as gps_pool, \
             tc.tile_pool(name="scps", bufs=3, space="PSUM") as sc_pool, \
             tc.tile_pool(name="otps", bufs=1, space="PSUM") as ot_pool, \
             tc.tile_pool(name="ppps", bufs=2, space="PSUM") as pp_pool:

            # --- initial loads. A DMA issued on an engine serializes that
            # engine's later instructions behind the transfer (completion-
            # semaphore rotation), so: scalar gets almost nothing (it must
            # run g01 copies + exps from ~35us), sync's loads must drain
            # before the chunk-0 partial stores (~55us), gpsimd's before
            # the first RS doorbell (plenty of slack — the RS chain only
            # has to stay ahead of the much later compute tail). ---
            wqkv_sb = []          # 8 granules of [P, 2, 768] (e = 2t, 2t+1)
            hid_sb = []           # per j: [P, 16, 512]; 3 rotating buffers
            for j in range(NQ):
                hid_sb.append(hid_pool.tile([P, NEC, 512], bt, name=f"hid{j}",
                                            tag="hid"))
            PAIR_ENG = [nc.sync, nc.sync, nc.sync, nc.sync,
                        nc.scalar, nc.scalar, nc.gpsimd, nc.gpsimd]
            for t in range(8):
                wt = wts_pool.tile([P, 2, 768], bt, name=f"wqkv_{t}",
                                   tag=f"wqkv_{t}")
                eng = PAIR_ENG[t]
                if t == 0:
                    # split the critical first granule so the very first
                    # group matmul can start after ~320KB, not ~640KB
                    eng.dma_start(wt[:, 0, :], wqkvT[0:P, :])
                    eng.dma_start(hid_sb[0][:, 0, :], hiddenT[0:P, 0:512])
                    eng.dma_start(wt[:, 1, :], wqkvT[P:2 * P, :])
                    eng.dma_start(hid_sb[0][:, 1, :], hiddenT[P:2 * P, 0:512])
                else:
                    eng.dma_start(
                        wt[:], wqkvT[256 * t:256 * (t + 1), :].rearrange(
                            "(o p) c -> p o c", p=P))
                    eng.dma_start(
                        hid_sb[0][:, 2 * t:2 * t + 2, :],
                        hiddenT[256 * t:256 * (t + 1), 0:512].rearrange(
                            "(o p) s -> p o s", p=P))
                wqkv_sb.append(wt)

            cos_sb = ph1c_pool.tile([P, S], bt, name="cos_sb", tag="cos_sb")
            sin_sb = ph1c_pool.tile([P, S], bt, name="sin_sb", tag="sin_sb")
            nc.scalar.dma_start(sin_sb[:], sinT)
            nc.gpsimd.dma_start(mask_sb[:], masksT)
            nc.scalar.dma_start(cos_sb[:], cosT)
            nc.gpsimd.dma_start(wo_sb[:], woutT.rearrange(
                "p (o e) -> p o e", o=NEC))

            # hid quarters: j1/j2 up front split sync/gpsimd; j3's gpsimd
            # share is emitted inside the j-loop after the first doorbell
            # (it must wait for hid buffer reuse, which would head-of-line
            # block the doorbell if queued before it).
            def emit_quarter(j, q, eng):
                eng.dma_start(
                    hid_sb[j][:, 4 * q:4 * q + 4, :],
                    hiddenT[512 * q:512 * (q + 1),
                            512 * j:512 * (j + 1)].rearrange(
                        "(o p) s -> p o s", p=P))

            for j in (1, 2):
                for q in range(4):
                    emit_quarter(j, q, nc.sync if q < 2 else nc.gpsimd)
            emit_quarter(3, 0, nc.sync)
            emit_quarter(3, 1, nc.sync)

            def wq_sl(e, cols):   # wqkv rows for contraction chunk e
                return wqkv_sb[e // 2][:, e % 2, cols]

            # per-chunk activations: q/k transposed [d, 512]; v [k-in-tile,
            # 4*128]
            qc = [[ck_pool.tile([P, 512], bt, name=f"qc{h}_{j}", tag=f"qc{h}_{j}")
                   for j in range(NQ)] for h in range(2)]
            kc = [[ck_pool.tile([P, 512], bt, name=f"kc{h}_{j}", tag=f"kc{h}_{j}")
                   for j in range(NQ)] for h in range(2)]
            vc = [[ck_pool.tile([P, 512], bt, name=f"vc{h}_{j}", tag=f"vc{h}_{j}")
                   for j in range(NQ)] for h in range(2)]

            def emit_attention(s):
                qlo, w = SUBS[s]
                j = qlo // 512
                co = qlo % 512          # column offset inside the qc/kc tile
                nk = (qlo + w) // P     # number of k tiles
                LOOK = 3
                for hi in range(2):
                    otp = ot_pool.tile([P, 512], f32, name=f"otp{hi}_{s}",
                                       tag="otp")
                    dacc = dac_pool.tile([P, 512], bt, name=f"dac{hi}_{s}",
                                         tag="dac")
                    pts = {}

                    def emit_score(i):
                        delta = P * i - qlo
                        off = max(0, delta)
                        scp = sc_pool.tile([P, 512], f32, name=f"scp{hi}{s}{i}",
                                           tag="scp")
                        nc.tensor.matmul(
                            scp[:, off:w], kc[hi][i // 4][:, (i % 4) * P:
                                                          (i % 4 + 1) * P],
                            qc[hi][j][:, co + off:co + w],
                            start=True, stop=True)
                        pt = pt_pool.tile([P, 512], bt, name=f"pt{hi}{s}{i}",
                                          tag="pt")
                        nc.scalar.activation(
                            pt[:, off:w], scp[:, off:w],
                            mybir.ActivationFunctionType.Exp, scale=SCALE)
                        if delta >= 0:
                            # only the first 128 cols of a diagonal tile can
                            # be masked; beyond that the tile is all-causal
                            mm = delta // P
                            mw = min(w, off + P)
                            nc.vector.tensor_mul(
                                pt[:, off:mw], pt[:, off:mw],
                                mask_sb[:, mm * 512 + off:mm * 512 + mw])
                        pts[i] = (pt, off)

                    def emit_pv(i):
                        pt, off = pts.pop(i)
                        nc.tensor.matmul(
                            otp[:, off:w], vc[hi][i // 4][:, (i % 4) * P:
                                                          (i % 4 + 1) * P],
                            pt[:, off:w],
                            start=(i == 0), stop=(i == nk - 1))
                        if i == 0:
                            nc.vector.tensor_copy(dacc[:, 0:w], pt[:, 0:w])
                        else:
                            nc.vector.tensor_add(dacc[:, off:w], dacc[:, off:w],
                                                 pt[:, off:w])

                    for i in range(min(LOOK, nk)):
                        emit_score(i)
                    for i in range(nk):
                        if i + LOOK < nk:
                            emit_score(i + LOOK)
                        emit_pv(i)

                    dnp = sc_pool.tile([P, 512], f32, name=f"dnp{hi}_{s}",
                                       tag="scp")
                    nc.tensor.matmul(dnp[:, 0:w], ones_sb[:], dacc[:, 0:w],
                                     start=True, stop=True)
                    den_sb = den_pool.tile([P, 512], f32, name=f"den{hi}_{s}",
                                           tag="den")
                    nc.vector.reciprocal_approx_fast(den_sb[:, 0:w],
                                                     dnp[:, 0:w])
                    otn = otn_pool.tile([P, 512], bt, name=f"otn{hi}_{s}",
                                        tag="otn")
                    nc.vector.tensor_mul(otn[:, 0:w], otp[:, 0:w],
                                         den_sb[:, 0:w])
                    # tiny store: this core's normalized O^T slice for the
                    # AllGather. On scalar — its queue stays thin, so the
                    # doorbell gate (these stores) is never stuck behind
                    # bulk loads or AG-completion waits.
                    hw = w // 2
                    nc.scalar.dma_start(ag_in[s][hi * P:(hi + 1) * P, 0:hw],
                                        otn[:, 0:hw])
                    nc.scalar.dma_start(ag_in[s][hi * P:(hi + 1) * P, hw:w],
                                        otn[:, hw:w])

            def emit_ag(s):
                nc.gpsimd.collective_compute(
                    "AllGather",
                    mybir.AluOpType.bypass,
                    replica_groups=[list(range(NCORES))],
                    ins=[ag_in[s].opt()],
                    outs=[ag_out[s].opt()],
                )

            oti_sb = {}

            def emit_oti_load(s):
                # gathered O^T chunk as [128, 16, w]; halves on gpsimd
                # (riding just behind a doorbell — the AG_s-completion wait
                # they carry is free, the CC stream serializes AGs anyway)
                # and sync.
                qlo, w = SUBS[s]
                tag = f"oti{s}"
                oti = oti_pool.tile([P, NEC, w], bt, name=f"oti{s}", tag=tag,
                                    bufs=1)
                src = ag_out[s].rearrange("(o p) s -> p o s", p=P)
                for t in range(4):
                    eng = nc.gpsimd if t % 2 == 0 else nc.sync
                    eng.dma_start(oti[:, 4 * t:4 * t + 4, :],
                                  src[:, 4 * t:4 * t + 4, :])
                oti_sb[s] = oti

            def emit_oproj_group(ss):
                # local out-proj for a GROUP of chunks: each wo stationary
                # tile is loaded once and reused for every chunk in the
                # group (attention is fully done by now, so the attention
                # psum pools are free to hold the extra accumulators).
                pools = {0: pp_pool, 1: gps_pool, 2: sc_pool}
                tags = {0: "pp", 1: "gps", 2: "scp"}
                pps = {}
                for k, s in enumerate(ss):
                    pps[s] = [pools[k].tile([P, 512], f32, name=f"pps{s}_{b}",
                                            tag=tags[k]) for b in range(2)]
                for fc in range(NEC):
                    for b in range(2):
                        for s in ss:
                            w = SUBS[s][1]
                            nc.tensor.matmul(
                                pps[s][b][:, 0:w],
                                wo_sb[:, fc, b * P:(b + 1) * P],
                                oti_sb[s][:, fc, :], start=(fc == 0),
                                stop=(fc == NEC - 1))
                for s in ss:
                    qlo, w = SUBS[s]
                    del oti_sb[s]
                    for b in range(2):
                        ob = psb_pool.tile([P, 512], bt, name=f"ob{s}_{b}",
                                           tag="ob")
                        if b == 0:
                            nc.scalar.copy(ob[:, 0:w], pps[s][b][:, 0:w])
                        else:
                            nc.vector.tensor_copy(ob[:, 0:w], pps[s][b][:, 0:w])
                        nc.scalar.dma_start(
                            outT[b * P:(b + 1) * P, qlo:qlo + w], ob[:, 0:w])

            for j in range(NQ):
                qs = slice(512 * j, 512 * (j + 1))
                # ---- qkv chunk j: rot groups in 2 waves of 2 psum banks ----
                g01 = []
                for g in (0, 1):
                    gp = gps_pool.tile([P, 512], f32, name=f"gps{j}_{g}",
                                       tag="gps")
                    for e in range(NEC):
                        nc.tensor.matmul(
                            gp[:], wq_sl(e, slice(g * P, (g + 1) * P)),
                            hid_sb[j][:, e, :], start=(e == 0),
                            stop=(e == NEC - 1))
                    gc = g01_pool.tile([P, 512], bt, name=f"g01_{j}_{g}",
                                       tag=f"g01_{g}")
                    nc.scalar.copy(gc[:], gp[:])
                    g01.append(gc)
                for g in (2, 3):
                    gp = gps_pool.tile([P, 512], f32, name=f"gps{j}_{g}",
                                       tag="gps")
                    for e in range(NEC):
                        nc.tensor.matmul(
                            gp[:], wq_sl(e, slice(g * P, (g + 1) * P)),
                            hid_sb[j][:, e, :], start=(e == 0),
                            stop=(e == NEC - 1))
                    dst = qc if g == 2 else kc
                    nc.vector.tensor_copy(dst[0][j][64:128, :], gp[0:64, :])
                    nc.vector.tensor_copy(dst[1][j][64:128, :], gp[64:128, :])
                # v chunk j
                for st in range(4):
                    vp = gps_pool.tile([P, 256], f32, name=f"vps{j}_{st}",
                                       tag="gps")
                    for e in range(NEC):
                        nc.tensor.matmul(
                            vp[:], hid_sb[j][:, e, st * P:(st + 1) * P],
                            wq_sl(e, slice(512, 768)), start=(e == 0),
                            stop=(e == NEC - 1))
                    nc.vector.tensor_copy(vc[0][j][:, st * P:(st + 1) * P],
                                          vp[:, 0:P])
                    nc.vector.tensor_copy(vc[1][j][:, st * P:(st + 1) * P],
                                          vp[:, P:2 * P])
                # rope chunk j
                t0 = rtmp_pool.tile([P, 512], bt, name=f"t0_{j}", tag="t0")
                t1 = rtmp_pool.tile([P, 512], bt, name=f"t1_{j}", tag="t1")
                ta = rtmp_pool.tile([P, 512], bt, name=f"ta_{j}", tag="ta")
                tb = rtmp_pool.tile([P, 512], bt, name=f"tb_{j}", tag="tb")
                nc.vector.tensor_mul(t0[:], g01[0][:], cos_sb[:, qs])
                nc.vector.tensor_mul(t1[:], g01[1][:], sin_sb[:, qs])
                nc.vector.tensor_sub(ta[:], t0[:], t1[:])      # tops
                nc.vector.tensor_mul(t0[:], g01[1][:], cos_sb[:, qs])
                nc.vector.tensor_mul(t1[:], g01[0][:], sin_sb[:, qs])
                nc.vector.tensor_add(tb[:], t0[:], t1[:])      # bottoms
                for pc, dst in enumerate((qc[0], qc[1], kc[0], kc[1])):
                    ps = slice(32 * pc, 32 * (pc + 1))
                    nc.vector.tensor_copy(dst[j][0:32, :], ta[ps, :])
                    nc.vector.tensor_copy(dst[j][32:64, :], tb[ps, :])

                # ---- attention + AG for this chunk. ALL out-projs are
                # deferred to after the last attention chunk: attn4 gates
                # the final AGs on every core (via the slowest core), so
                # finishing attention ASAP shortens the whole kernel; the
                # deferred oproj work then overlaps the final AG latency.
                for s in SUBS_OF_J[j]:
                    emit_attention(s)
                    emit_ag(s)
                    if j == 0:
                        # j3's gpsimd hid quarters ride after the first
                        # doorbell (they wait on hid buffer reuse)
                        emit_quarter(3, 2, nc.gpsimd)
                        emit_quarter(3, 3, nc.gpsimd)
                    if s >= 1:
                        emit_oti_load(s - 1)

            emit_oti_load(len(SUBS) - 1)
            emit_oproj_group([0, 1, 2])
            emit_oproj_group([3, 4])


def _build_program():
    import concourse.bass as bass  # noqa: F401
    import concourse.mybir as mybir
    import concourse.tile as tile
    from concourse import bacc

    nc = bacc.Bacc("TRN2", target_bir_lowering=False, debug=False,
                   enable_asserts=False, num_devices=NCORES)
    bt = mybir.dt.bfloat16
    hiddenT = nc.dram_tensor("hiddenT", [E, S], bt, kind="ExternalInput").ap()
    wqkvT = nc.dram_tensor("wqkvT", [E, 768], bt, kind="ExternalInput").ap()
    woutT = nc.dram_tensor("woutT", [P, 4096], bt, kind="ExternalInput").ap()
    cosT = nc.dram_tensor("cosT", [P, S], bt, kind="ExternalInput").ap()
    sinT = nc.dram_tensor("sinT", [P, S], bt, kind="ExternalInput").ap()
    masks = nc.dram_tensor("masks", [P, 4 * 512], bt, kind="ExternalInput").ap()
    outT = nc.dram_tensor("outT", [2 * P, S], bt, kind="ExternalOutput").ap()

    with tile.TileContext(nc) as tc:
        _kernel_body(tc, outT, hiddenT, wqkvT, woutT, cosT, sinT, masks)
    nc.compile()
    return nc


def get_program():
    if "nc" not in _CACHE:
        _CACHE["nc"] = _build_program()
    return _CACHE["nc"]


def _install_ntff_shim():
    """Provide antenv.axon_hooks (missing in this image) so trace=True works."""
    import sys
    import types
    try:
        import antenv.axon_hooks  # noqa: F401
        return
    except ImportError:
        pass
    import antenv
    mod = types.ModuleType("antenv.axon_hooks")
    mod._hook = None

    def set_axon_ntff_profile_hook(h):
        mod._hook = h

    def get_axon_ntff_profile_hook():
        return mod._hook

    mod.set_axon_ntff_profile_hook = set_axon_ntff_profile_hook
    mod.get_axon_ntff_profile_hook = get_axon_ntff_profile_hook
    sys.modules["antenv.axon_hooks"] = mod
    antenv.axon_hooks = mod
    try:
        from trn_agent_boot.trn_boot import _ntff_profile_via_ctypes
        hook = _ntff_profile_via_ctypes("/opt/axon/libaxon_pjrt.so")
        if hook is not None:
            mod._hook = hook
    except Exception:
        pass


def run(inputs, trace=False):
    """Run on the 8 NeuronCores; returns (out [1,S,E], BassKernelResults)."""
    from concourse import bass_utils

    if trace:
        _install_ntff_shim()
    nc = get_program()
    in_maps = _build_in_maps(inputs["hidden_states"], inputs["w_qkv"],
                             inputs["w_out"])
    res = bass_utils.run_bass_kernel_spmd(
        nc, in_maps, core_ids=list(range(NCORES)), trace=trace)
    outT = np.concatenate(
        [np.asarray(res.results[c]["outT"]).astype(np.float32)
         for c in range(NCORES)], axis=0)  # [E, S]
    out = np.ascontiguousarray(outT.T).reshape(1, S, E).astype(np.float32)
    return out, res


def kernel(hidden_states, w_qkv, w_out):
    out, _ = run({"hidden_states": hidden_states, "w_qkv": w_qkv,
                  "w_out": w_out})
    return out
